# revision 61
# baseline (speedup 1.0000x reference)
"""AvgPool2d(16x16, stride 1, count_include_pad-style box sums) with
replicate-padded output, as a Bass/Tile kernel for 8 Trainium2 NeuronCores.

Input  x: (4, 64, 512, 512) fp32  -> 256 independent (n,c) planes.
Output: same shape; out = replicate_pad(avg_pool_valid(x)), per reference:
  box[h', w'] = sum_{i<16, j<16} x[h'+i, w'+j] / 256, h',w' in [0, 497)
  out[hp, wp] = box[clamp(hp-7, 0, 496), clamp(wp-7, 0, 496)]

Per-core algorithm (32 planes/core, data-parallel over planes, no comms):
  1. DMA plane rows in 4 chunks of 128 rows (one DMA per chunk): xt
     [128, 4, 512] fp32, rows rolled by +7 so each 128-row output group
     needs exactly two contraction chunks.
  2. W-direction sliding window-16 sum on VectorE:
       box_w[0] = reduce_sum(x[0:16]);
       scan j=1..496: state = (x[j+15] + state) - x[j-1]   (tensor_tensor_scan)
     The scan state stays fp32 internally; the OUTPUT is written fp16
     (one rounding per element, no error accumulation) -> bw [128, 512] f16.
  3. H-direction window sum + 1/256 scale + H-replicate-pad folded into one
     banded-matrix matmul on TensorE: out[hp, w'] = sum_h B[h, hp] * bw[h, w']
     with B and bw in fp16 (native single-pass matmul, 1/256 exact in fp16).
  4. ScalarE evacuates PSUM -> fp16 SBUF and writes W-replicate-pad columns
     via activation(Identity, scale=0, bias=edge_column) broadcasts; for the
     last two planes VectorE (idle after the final scans) evacuates instead
     and stores go out per chunk, shortening the serial drain ~10us.
  5. fp16 stores from the ACT sequencer (half the store bytes; host upcasts
     to fp32). Output rounding adds ~1.4e-4 rel; total rel err ~3.6e-4
     (gate is 2e-2).

Performance notes (measured on TRN2 via NTFF traces, 8 cores; baseline
fp32 version was 244us):
- Device clock varies ~20% run-to-run (scan slice 1150ns vs 1380-1500ns);
  compare runs via per-op medians, not wall time. Best measured 200270ns
  (fast clock); mid-clock runs ~204-207us (~189us fast-equivalent).
- VectorE is the pacer: 128 scans x ~1.15us (2.3ns/elem) + 128 reduces
  ~= 162us busy, >90% occupancy inside its window. 16-bit operands do
  NOT speed up DVE on HW (tensor_add tree measured 0.78ns/elem both f32
  and f16 -> 4-pass tree loses to the fused scan). A zero-padded
  initial=0 scan that kills the reduces saves ~7us of DVE busy but adds
  ~20us of new cross-engine wait bubbles - net loss, not used.
- DMA: store descriptors cost a fixed ~80ns per DRAM row line regardless
  of line size (fp16 halves bytes, not lines); loads ~43ns/line. A
  single DMA instruction only sustains ~90GB/s (descriptors land on ~3
  of 16 queues), hence per-chunk loads: chunk c's scan gates on its own
  256KiB (~3us) instead of a whole-plane load (~8.3us). wt loads go via
  GPSIMD's SWDGE to keep SP/ACT free during the ramp.
- Issue placement is critical: stores MUST be issued from the ACT
  sequencer (same engine as evac/pads). SP-issued stores stall the load
  chain on the pad wait (+44us); per-m stores from ACT congest its
  sequencer (+15us). fp32r matmul works (s3d3: widen rhs/out to a
  4-aligned N=500) but fp16 weights+rhs is simpler and equally fast;
  with fp16 matmuls walrus's --enable-ldw-opt must stay false (explicit
  Ldweights pairs are incompatible).
- bw pool 24 bufs (6 planes) + a DVE absorber op observing mm(p-6) keeps
  scans' waits at <=1; deeper pools absorb PE lag during pipeline fill.
"""
import numpy as np
from contextlib import ExitStack

import concourse.bass as bass
import concourse.bacc as bacc
import concourse.tile as tile
from concourse import mybir
from concourse.bass_utils import run_bass_kernel_spmd
from concourse.tile import add_dep_helper

NCORES = 8
N, C, H, W = 4, 64, 512, 512
K = 16
NW = H - K + 1        # 497 valid box positions per axis
PAD_T = (H - NW) // 2  # 7 (same for W)
PLANES = (N * C) // NCORES  # 32 planes per core
NCH = H // 128        # 4 row-chunks of 128


def _band_matrix() -> np.ndarray:
    """BT[h, hp] = 1/256 on the (clamped) band; lhsT layout for out = BT.T @ bw.

    Rolled by +PAD_T along h so each 128-row chunk c covers plane rows
    [128c-7, 128c+121) (chunk 0 wraps: rows 505..511 sit at partitions
    0..6). Each 128-row output group then needs exactly TWO contraction
    chunks: c=m and c=(m+1)%4."""
    bt = np.zeros((H, H), np.float32)
    for hp in range(H):
        lo = min(max(hp - PAD_T, 0), H - K)
        bt[lo:lo + K, hp] = 1.0 / (K * K)
    return np.roll(bt, PAD_T, axis=0)


def _k_chunks(bt: np.ndarray) -> list[list[int]]:
    ks = []
    for m in range(NCH):
        ks.append([c for c in range(NCH)
                   if np.any(bt[128 * c:128 * (c + 1), 128 * m:128 * (m + 1)])])
    return ks


def _build_program(planes: int = PLANES):
    f32 = mybir.dt.float32
    f32r = mybir.dt.float32r
    f16 = mybir.dt.float16
    bt_np = _band_matrix()
    ks_per_m = _k_chunks(bt_np)

    nc = bacc.Bacc("TRN2", target_bir_lowering=False, debug=False,
                   num_devices=NCORES, num_swdge_queues=4)
    x_ap = nc.dram_tensor("x", [planes, H, W], f32, kind="ExternalInput").ap()
    # Band matrix passed from the host pre-cast to fp16 (1/256 is exact).
    bt_ap = nc.dram_tensor("bt", [H, H], f16, kind="ExternalInput").ap()
    # fp16 output: halves store traffic (the binding DMA/HBM cost); the
    # host upcasts back to fp32. fp16 rounding adds ~1.4e-4 rel error.
    o_ap = nc.dram_tensor("out", [planes, H, W], f16, kind="ExternalOutput").ap()

    with tile.TileContext(nc) as tc, ExitStack() as ctx:
        wpool = ctx.enter_context(tc.tile_pool(name="wt", bufs=1))
        xpool = ctx.enter_context(tc.tile_pool(name="xt", bufs=10))
        bwpool = ctx.enter_context(tc.tile_pool(name="bw", bufs=24))
        opool = ctx.enter_context(tc.tile_pool(name="osb", bufs=8))
        pspool = ctx.enter_context(tc.tile_pool(name="ps", bufs=8, space="PSUM"))

        # --- weights: 4 chunks of rolled BT rows -> [128 (h), 512 (hp)] ---
        wt = []
        wt_dma = []
        for c in range(NCH):
            t = wpool.tile([128, H], f16, tag=f"wt{c}")
            # Issue the 0.5 MiB band-matrix load from the idle GPSIMD's
            # SWDGE: keeps both the SP chain (x loads) and ACT free during
            # the ramp; wt is not needed until the first matmul (~20us).
            wt_dma.append(nc.gpsimd.dma_start(
                t, bt_ap[128 * c:128 * (c + 1), :]))
            wt.append(t)
        # Dummy matmuls make the PE proc observe the weight-DMA queue sems
        # up front so real matmuls don't need event-sem carried weight waits.
        scratch = pspool.tile([2, 2], f32, tag="pt")
        wt_guards = [
            nc.tensor.matmul(scratch[:, :], lhsT=wt[c][:, 0:2],
                             rhs=wt[c][:, 0:2], start=True,
                             stop=True, skip_group_check=True)
            for c in range(NCH)
        ]
        # tiny per-engine scratch tiles for wait-absorber ops
        dve_scr = wpool.tile([1, 4], f32, tag="dve_scr")
        act_scr = wpool.tile([1, 4], f32, tag="act_scr")


        # Ordering-only pins keep the HWDGE round-robin phase stable-ish.
        dma_chain = []

        def chain(inst):
            if dma_chain:
                add_dep_helper(inst.ins, dma_chain[-1].ins, sync=False,
                               reason="pin HWDGE round-robin phase")
            dma_chain.append(inst)

        # Chunk-0 loads are issued one plane AHEAD of chunks 1-3 (chain
        # order per plane: c1(p), c2(p), c3(p), c0(p+1)): chunk 0 is the
        # first one the scans consume, so giving it a full plane period
        # of lead hides the ~1.3us DMA completion latency that otherwise
        # shows up as a per-plane DVE gap during pipeline fill.
        xt_tiles = {}

        def load_c0(p):
            xt = xpool.tile([128, NCH, W], f32)
            chain(nc.sync.dma_start(xt[PAD_T:128, 0, :], x_ap[p, 0:121, :]))
            chain(nc.sync.dma_start(xt[0:PAD_T, 0, :],
                                    x_ap[p, H - PAD_T:H, :]))
            xt_tiles[p] = xt

        load_c0(0)
        # plane 0's chunk 1 too: hides its DMA latency behind chunk 0's scan
        chain(nc.sync.dma_start(xt_tiles[0][:, 1, :],
                                x_ap[0, 128 - PAD_T:128 + 121, :]))
        last_mm = {}
        prev_last_scan = None
        pinned = False
        for p in range(planes):
            # DVE absorber: observe the PE tick that frees this plane's bw
            # slots (bufs=24 -> plane p-6's last matmul) so the reduces only
            # carry their xt-DMA wait.
            dve_abs = None
            if p - 6 in last_mm:
                dve_abs = nc.vector.tensor_copy(dve_scr[:, :], dve_scr[:, :])
                add_dep_helper(dve_abs.ins, last_mm[p - 6].ins,
                               reason="DVE observes bw slot release")
                if prev_last_scan is not None:
                    # Without this pin the scheduler hoists the absorber
                    # ~5 planes early in the DVE stream, where its
                    # mm(p-6) wait stalls DVE ~1.1us per plane while PE
                    # catches up during pipeline fill.
                    add_dep_helper(dve_abs.ins, prev_last_scan.ins,
                                   sync=False,
                                   reason="keep absorber at its plane")
            # One [128, 4*512] tile holds the whole plane with rows rolled
            # by +7: xt[q, c, :] = x[(128c + q - 7) mod 512, :]. Chunk 0
            # wraps (rows 505..511 at partitions 0..6). One DMA per chunk:
            # a single DMA instruction only sustains ~90 GB/s (descriptors
            # land on ~3 queues), so chunk-granular loads let chunk c's
            # scan start ~3us after its own 256 KiB instead of waiting
            # ~8.3us for a whole-plane load.
            xt = xt_tiles.pop(p)
            for c in range(2 if p == 0 else 1, NCH):
                chain(nc.sync.dma_start(
                    xt[:, c, :], x_ap[p, 128 * c - PAD_T:128 * c + 121, :]))
            if p + 1 < planes:
                load_c0(p + 1)
            bw = []
            for c in range(NCH):
                b = bwpool.tile([128, W], f16)
                # box_w[0]; also absorbs xt-DMA + bw-slot waits for the
                # scan. fp16 output: the scan state stays fp32 internally
                # and each output is rounded once (no error accumulation),
                # so fp16 bw costs ~2.6e-4 rel and feeds a native fp16
                # single-pass matmul.
                with nc.allow_low_precision("fp16 box sums; scan state "
                                            "stays fp32"):
                    rd = nc.vector.reduce_sum(b[:, K - 1:K], xt[:, c, 0:K],
                                              axis=mybir.AxisListType.X)
                    if dve_abs is not None:
                        add_dep_helper(rd.ins, dve_abs.ins, sync=False,
                                       reason="pin reduce after DVE absorber")
                    sc = nc.vector.tensor_tensor_scan(
                        out=b[:, K:W],
                        data0=xt[:, c, K:W],
                        data1=xt[:, c, 0:W - K],
                        initial=b[:, K - 1:K],
                        op0=mybir.AluOpType.add,
                        op1=mybir.AluOpType.subtract,
                    )
                bw.append(b)
            prev_last_scan = sc

            # ACT absorber: observe the out-DMA that frees this plane's osb
            # slot (bufs=4) so evacuations only carry their PE wait.

            osb = opool.tile([128, NCH, W], f16)
            for m in range(NCH):
                pt = pspool.tile([128, W], f32, tag="pt")
                ks = ks_per_m[m]
                for i, c in enumerate(ks):
                    # native fp16 single-pass matmul (2-byte weights use
                    # one PE weight buffer, so LDWEIGHTS overlaps).
                    mm = nc.tensor.matmul(
                        pt[:, PAD_T:PAD_T + NW],
                        lhsT=wt[c][:, 128 * m:128 * (m + 1)],
                        rhs=bw[c][:, K - 1:W],
                        start=(i == 0),
                        stop=(i == len(ks) - 1),
                    )
                    if not pinned:
                        pinned = True
                        for g in wt_guards:
                            add_dep_helper(mm.ins, g.ins, sync=False,
                                           reason="pin MMs after wt guards")
                last_mm[p] = mm

                tail = p >= planes - 2
                with nc.allow_low_precision("fp16 output stores"):
                    if tail:
                        # Last two planes: DVE is idle after the final
                        # scans -- let it evacuate psum while ACT pads the
                        # previous chunk, and store per chunk, shortening
                        # the serial drain by ~10us.
                        nc.vector.tensor_copy(osb[:, m, PAD_T:PAD_T + NW],
                                              pt[:, PAD_T:PAD_T + NW])
                    else:
                        nc.scalar.copy(osb[:, m, PAD_T:PAD_T + NW],
                                       pt[:, PAD_T:PAD_T + NW])
                # W replicate-pad on ACT (bias broadcasts): keeps the
                # whole evac -> edges -> store chain on one engine with no
                # cross-engine semaphores.
                nc.scalar.activation(
                    osb[:, m, 0:PAD_T], osb[:, m, PAD_T:2 * PAD_T],
                    mybir.ActivationFunctionType.Identity,
                    bias=osb[:, m, PAD_T:PAD_T + 1], scale=0.0)
                nc.scalar.activation(
                    osb[:, m, PAD_T + NW:W], osb[:, m, NW - 1:NW + PAD_T],
                    mybir.ActivationFunctionType.Identity,
                    bias=osb[:, m, PAD_T + NW - 1:PAD_T + NW], scale=0.0)
                if tail:
                    nc.scalar.dma_start(o_ap[p, 128 * m:128 * (m + 1), :],
                                        osb[:, m, :])
            if p < planes - 2:
                o_view = o_ap[p].rearrange("(m q) w -> q m w", q=128)
                # Issue stores from the ACT sequencer: same-engine with
                # the evac/pads, so the issue never blocks another
                # engine's chain (SP-issued stores stall the load chain
                # on the pad wait).
                nc.scalar.dma_start(o_view, osb[:, :, :])

    nc.compile()
    return nc


_NC_CACHE = {}


def _get_nc(planes: int = PLANES):
    if planes not in _NC_CACHE:
        _NC_CACHE[planes] = _build_program(planes)
    return _NC_CACHE[planes]


def run_sharded(x: np.ndarray, trace: bool = False, trace_cores=None, **kw):
    """x: (N, C, H, W) fp32 -> (out (N,C,H,W) fp32, BassKernelResults)."""
    nc = _get_nc()
    planes_all = np.ascontiguousarray(x.reshape(N * C, H, W), dtype=np.float32)
    bt_np = _band_matrix().astype(np.float16)  # 1/256 is exact in fp16
    in_maps = [
        {"x": planes_all[i * PLANES:(i + 1) * PLANES], "bt": bt_np}
        for i in range(NCORES)
    ]
    r = run_bass_kernel_spmd(nc, in_maps, list(range(NCORES)),
                             trace=trace, trace_cores=trace_cores, **kw)
    out = np.concatenate([r.results[i]["out"] for i in range(NCORES)], axis=0)
    return out.reshape(N, C, H, W).astype(np.float32), r


def kernel(x: np.ndarray) -> np.ndarray:
    out, _ = run_sharded(np.asarray(x))
    return out


if __name__ == "__main__":
    # quick compile-only probe with a reduced plane count
    import sys
    import tempfile
    from concourse.bass_utils import compile_bir_kernel

    planes = int(sys.argv[1]) if len(sys.argv) > 1 else 2
    nc = _build_program(planes)
    d = tempfile.mkdtemp()
    print(f"compiling {planes}-plane program to {d} ...")
    neff = compile_bir_kernel(nc.to_json_bytes(), d, neff_name="probe.neff")
    print(f"COMPILE OK: {neff}")



# revision 63
# speedup vs baseline: 1.0379x; 1.0379x over previous
"""AvgPool2d(16x16, stride 1, count_include_pad-style box sums) with
replicate-padded output, as a Bass/Tile kernel for 8 Trainium2 NeuronCores.

Input  x: (4, 64, 512, 512) fp32  -> 256 independent (n,c) planes.
Output: same shape; out = replicate_pad(avg_pool_valid(x)), per reference:
  box[h', w'] = sum_{i<16, j<16} x[h'+i, w'+j] / 256, h',w' in [0, 497)
  out[hp, wp] = box[clamp(hp-7, 0, 496), clamp(wp-7, 0, 496)]

Per-core algorithm (32 planes/core, data-parallel over planes, no comms):
  1. DMA plane rows in 4 chunks of 128 rows (one DMA per chunk): xt
     [128, 4, 512] fp32, rows rolled by +7 so each 128-row output group
     needs exactly two contraction chunks.
  2. W-direction sliding window-16 sum on VectorE:
       box_w[0] = reduce_sum(x[0:16]);
       scan j=1..496: state = (x[j+15] + state) - x[j-1]   (tensor_tensor_scan)
     The scan state stays fp32 internally; the OUTPUT is written fp16
     (one rounding per element, no error accumulation) -> bw [128, 512] f16.
  3. H-direction window sum + 1/256 scale + H-replicate-pad folded into one
     banded-matrix matmul on TensorE: out[hp, w'] = sum_h B[h, hp] * bw[h, w']
     with B and bw in fp16 (native single-pass matmul, 1/256 exact in fp16).
  4. ScalarE evacuates PSUM -> fp16 SBUF and writes W-replicate-pad columns
     via activation(Identity, scale=0, bias=edge_column) broadcasts; for the
     last two planes VectorE (idle after the final scans) evacuates instead
     and stores go out per chunk, shortening the serial drain ~10us.
  5. fp16 stores from the ACT sequencer (half the store bytes; host upcasts
     to fp32). Output rounding adds ~1.4e-4 rel; total rel err ~3.6e-4
     (gate is 2e-2).

Performance notes (measured on TRN2 via NTFF traces, 8 cores; baseline
fp32 version was 244us):
- Device clock varies ~20% run-to-run (scan slice 1150ns vs 1380-1500ns);
  compare runs via per-op medians, not wall time. Best measured 200270ns
  (fast clock); mid-clock runs ~204-207us (~189us fast-equivalent).
- VectorE is the pacer: 128 scans x ~1.15us (2.3ns/elem) + 128 reduces
  ~= 162us busy, >90% occupancy inside its window. 16-bit operands do
  NOT speed up DVE on HW (tensor_add tree measured 0.78ns/elem both f32
  and f16 -> 4-pass tree loses to the fused scan). A zero-padded
  initial=0 scan that kills the reduces saves ~7us of DVE busy but adds
  ~20us of new cross-engine wait bubbles - net loss, not used.
- DMA: store descriptors cost a fixed ~80ns per DRAM row line regardless
  of line size (fp16 halves bytes, not lines); loads ~43ns/line. A
  single DMA instruction only sustains ~90GB/s (descriptors land on ~3
  of 16 queues), hence per-chunk loads: chunk c's scan gates on its own
  256KiB (~3us) instead of a whole-plane load (~8.3us). wt loads go via
  GPSIMD's SWDGE to keep SP/ACT free during the ramp.
- Issue placement is critical: stores MUST be issued from the ACT
  sequencer (same engine as evac/pads). SP-issued stores stall the load
  chain on the pad wait (+44us); per-m stores from ACT congest its
  sequencer (+15us). fp32r matmul works (s3d3: widen rhs/out to a
  4-aligned N=500) but fp16 weights+rhs is simpler and equally fast;
  with fp16 matmuls walrus's --enable-ldw-opt must stay false (explicit
  Ldweights pairs are incompatible).
- bw pool 24 bufs (6 planes) + a DVE absorber op observing mm(p-6) keeps
  scans' waits at <=1; deeper pools absorb PE lag during pipeline fill.
"""
import numpy as np
from contextlib import ExitStack

import concourse.bass as bass
import concourse.bacc as bacc
import concourse.tile as tile
from concourse import mybir
from concourse.bass_utils import run_bass_kernel_spmd
from concourse.tile import add_dep_helper

NCORES = 8
N, C, H, W = 4, 64, 512, 512
K = 16
NW = H - K + 1        # 497 valid box positions per axis
PAD_T = (H - NW) // 2  # 7 (same for W)
PLANES = (N * C) // NCORES  # 32 planes per core
NCH = H // 128        # 4 row-chunks of 128


def _band_matrix() -> np.ndarray:
    """BT[h, hp] = 1/256 on the (clamped) band; lhsT layout for out = BT.T @ bw.

    Rolled by +PAD_T along h so each 128-row chunk c covers plane rows
    [128c-7, 128c+121) (chunk 0 wraps: rows 505..511 sit at partitions
    0..6). Each 128-row output group then needs exactly TWO contraction
    chunks: c=m and c=(m+1)%4."""
    bt = np.zeros((H, H), np.float32)
    for hp in range(H):
        lo = min(max(hp - PAD_T, 0), H - K)
        bt[lo:lo + K, hp] = 1.0 / (K * K)
    return np.roll(bt, PAD_T, axis=0)


def _k_chunks(bt: np.ndarray) -> list[list[int]]:
    ks = []
    for m in range(NCH):
        ks.append([c for c in range(NCH)
                   if np.any(bt[128 * c:128 * (c + 1), 128 * m:128 * (m + 1)])])
    return ks


def _build_program(planes: int = PLANES):
    f32 = mybir.dt.float32
    f32r = mybir.dt.float32r
    f16 = mybir.dt.float16
    bt_np = _band_matrix()
    ks_per_m = _k_chunks(bt_np)

    nc = bacc.Bacc("TRN2", target_bir_lowering=False, debug=False,
                   num_devices=NCORES, num_swdge_queues=4)
    x_ap = nc.dram_tensor("x", [planes, H, W], f32, kind="ExternalInput").ap()
    # Band matrix passed from the host pre-cast to fp16 (1/256 is exact).
    bt_ap = nc.dram_tensor("bt", [H, H], f16, kind="ExternalInput").ap()
    # fp16 output: halves store traffic (the binding DMA/HBM cost); the
    # host upcasts back to fp32. fp16 rounding adds ~1.4e-4 rel error.
    o_ap = nc.dram_tensor("out", [planes, H, W], f16, kind="ExternalOutput").ap()

    with tile.TileContext(nc) as tc, ExitStack() as ctx:
        wpool = ctx.enter_context(tc.tile_pool(name="wt", bufs=1))
        xpool = ctx.enter_context(tc.tile_pool(name="xt", bufs=10))
        bwpool = ctx.enter_context(tc.tile_pool(name="bw", bufs=24))
        opool = ctx.enter_context(tc.tile_pool(name="osb", bufs=8))
        pspool = ctx.enter_context(tc.tile_pool(name="ps", bufs=8, space="PSUM"))

        # --- weights: 4 chunks of rolled BT rows -> [128 (h), 512 (hp)] ---
        wt = []
        wt_dma = []
        for c in range(NCH):
            t = wpool.tile([128, H], f16, tag=f"wt{c}")
            # Issue the 0.5 MiB band-matrix load from the idle GPSIMD's
            # SWDGE: keeps both the SP chain (x loads) and ACT free during
            # the ramp; wt is not needed until the first matmul (~20us).
            wt_dma.append(nc.gpsimd.dma_start(
                t, bt_ap[128 * c:128 * (c + 1), :]))
            wt.append(t)
        # Dummy matmuls make the PE proc observe the weight-DMA queue sems
        # up front so real matmuls don't need event-sem carried weight waits.
        scratch = pspool.tile([2, 2], f32, tag="pt")
        wt_guards = [
            nc.tensor.matmul(scratch[:, :], lhsT=wt[c][:, 0:2],
                             rhs=wt[c][:, 0:2], start=True,
                             stop=True, skip_group_check=True)
            for c in range(NCH)
        ]
        # tiny per-engine scratch tiles for wait-absorber ops
        dve_scr = wpool.tile([1, 4], f32, tag="dve_scr")
        act_scr = wpool.tile([1, 4], f32, tag="act_scr")


        # Ordering-only pins keep the HWDGE round-robin phase stable-ish.
        dma_chain = []

        def chain(inst):
            if dma_chain:
                add_dep_helper(inst.ins, dma_chain[-1].ins, sync=False,
                               reason="pin HWDGE round-robin phase")
            dma_chain.append(inst)

        # Chunk-0 loads are issued one plane AHEAD of chunks 1-3 (chain
        # order per plane: c1(p), c2(p), c3(p), c0(p+1)): chunk 0 is the
        # first one the scans consume, so giving it a full plane period
        # of lead hides the ~1.3us DMA completion latency that otherwise
        # shows up as a per-plane DVE gap during pipeline fill.
        xt_tiles = {}

        def load_c0(p):
            xt = xpool.tile([128, NCH, W], f32)
            chain(nc.sync.dma_start(xt[PAD_T:128, 0, :], x_ap[p, 0:121, :]))
            chain(nc.sync.dma_start(xt[0:PAD_T, 0, :],
                                    x_ap[p, H - PAD_T:H, :]))
            xt_tiles[p] = xt

        load_c0(0)
        last_mm = {}
        prev_last_scan = None
        pinned = False
        for p in range(planes):
            # DVE absorber: observe the PE tick that frees this plane's bw
            # slots (bufs=24 -> plane p-6's last matmul) so the reduces only
            # carry their xt-DMA wait.
            dve_abs = None
            if p - 6 in last_mm:
                dve_abs = nc.vector.tensor_copy(dve_scr[:, :], dve_scr[:, :])
                add_dep_helper(dve_abs.ins, last_mm[p - 6].ins,
                               reason="DVE observes bw slot release")
                if prev_last_scan is not None:
                    # Without this pin the scheduler hoists the absorber
                    # ~5 planes early in the DVE stream, where its
                    # mm(p-6) wait stalls DVE ~1.1us per plane while PE
                    # catches up during pipeline fill.
                    add_dep_helper(dve_abs.ins, prev_last_scan.ins,
                                   sync=False,
                                   reason="keep absorber at its plane")
            # One [128, 4*512] tile holds the whole plane with rows rolled
            # by +7: xt[q, c, :] = x[(128c + q - 7) mod 512, :]. Chunk 0
            # wraps (rows 505..511 at partitions 0..6). One DMA per chunk:
            # a single DMA instruction only sustains ~90 GB/s (descriptors
            # land on ~3 queues), so chunk-granular loads let chunk c's
            # scan start ~3us after its own 256 KiB instead of waiting
            # ~8.3us for a whole-plane load.
            xt = xt_tiles.pop(p)
            for c in range(1, NCH):
                chain(nc.sync.dma_start(
                    xt[:, c, :], x_ap[p, 128 * c - PAD_T:128 * c + 121, :]))
            if p + 1 < planes:
                load_c0(p + 1)
            bw = []
            for c in range(NCH):
                b = bwpool.tile([128, W], f16)
                # box_w[0]; also absorbs xt-DMA + bw-slot waits for the
                # scan. fp16 output: the scan state stays fp32 internally
                # and each output is rounded once (no error accumulation),
                # so fp16 bw costs ~2.6e-4 rel and feeds a native fp16
                # single-pass matmul.
                with nc.allow_low_precision("fp16 box sums; scan state "
                                            "stays fp32"):
                    rd = nc.vector.reduce_sum(b[:, K - 1:K], xt[:, c, 0:K],
                                              axis=mybir.AxisListType.X)
                    if dve_abs is not None:
                        add_dep_helper(rd.ins, dve_abs.ins, sync=False,
                                       reason="pin reduce after DVE absorber")
                    sc = nc.vector.tensor_tensor_scan(
                        out=b[:, K:W],
                        data0=xt[:, c, K:W],
                        data1=xt[:, c, 0:W - K],
                        initial=b[:, K - 1:K],
                        op0=mybir.AluOpType.add,
                        op1=mybir.AluOpType.subtract,
                    )
                bw.append(b)
            prev_last_scan = sc

            # ACT absorber: observe the out-DMA that frees this plane's osb
            # slot (bufs=4) so evacuations only carry their PE wait.

            osb = opool.tile([128, NCH, W], f16)
            for m in range(NCH):
                pt = pspool.tile([128, W], f32, tag="pt")
                ks = ks_per_m[m]
                for i, c in enumerate(ks):
                    # native fp16 single-pass matmul (2-byte weights use
                    # one PE weight buffer, so LDWEIGHTS overlaps).
                    mm = nc.tensor.matmul(
                        pt[:, PAD_T:PAD_T + NW],
                        lhsT=wt[c][:, 128 * m:128 * (m + 1)],
                        rhs=bw[c][:, K - 1:W],
                        start=(i == 0),
                        stop=(i == len(ks) - 1),
                    )
                    if not pinned:
                        pinned = True
                        for g in wt_guards:
                            add_dep_helper(mm.ins, g.ins, sync=False,
                                           reason="pin MMs after wt guards")
                last_mm[p] = mm

                tail = p >= planes - 2
                with nc.allow_low_precision("fp16 output stores"):
                    if tail:
                        # Last two planes: DVE is idle after the final
                        # scans -- let it evacuate psum while ACT pads the
                        # previous chunk, and store per chunk, shortening
                        # the serial drain by ~10us.
                        nc.vector.tensor_copy(osb[:, m, PAD_T:PAD_T + NW],
                                              pt[:, PAD_T:PAD_T + NW])
                    else:
                        nc.scalar.copy(osb[:, m, PAD_T:PAD_T + NW],
                                       pt[:, PAD_T:PAD_T + NW])
                # W replicate-pad on ACT (bias broadcasts): keeps the
                # whole evac -> edges -> store chain on one engine with no
                # cross-engine semaphores.
                nc.scalar.activation(
                    osb[:, m, 0:PAD_T], osb[:, m, PAD_T:2 * PAD_T],
                    mybir.ActivationFunctionType.Identity,
                    bias=osb[:, m, PAD_T:PAD_T + 1], scale=0.0)
                nc.scalar.activation(
                    osb[:, m, PAD_T + NW:W], osb[:, m, NW - 1:NW + PAD_T],
                    mybir.ActivationFunctionType.Identity,
                    bias=osb[:, m, PAD_T + NW - 1:PAD_T + NW], scale=0.0)
                if tail:
                    nc.scalar.dma_start(o_ap[p, 128 * m:128 * (m + 1), :],
                                        osb[:, m, :])
            if p < planes - 2:
                o_view = o_ap[p].rearrange("(m q) w -> q m w", q=128)
                # Issue stores from the ACT sequencer: same-engine with
                # the evac/pads, so the issue never blocks another
                # engine's chain (SP-issued stores stall the load chain
                # on the pad wait).
                nc.scalar.dma_start(o_view, osb[:, :, :])

    nc.compile()
    return nc


_NC_CACHE = {}


def _get_nc(planes: int = PLANES):
    if planes not in _NC_CACHE:
        _NC_CACHE[planes] = _build_program(planes)
    return _NC_CACHE[planes]


def run_sharded(x: np.ndarray, trace: bool = False, trace_cores=None, **kw):
    """x: (N, C, H, W) fp32 -> (out (N,C,H,W) fp32, BassKernelResults)."""
    nc = _get_nc()
    planes_all = np.ascontiguousarray(x.reshape(N * C, H, W), dtype=np.float32)
    bt_np = _band_matrix().astype(np.float16)  # 1/256 is exact in fp16
    in_maps = [
        {"x": planes_all[i * PLANES:(i + 1) * PLANES], "bt": bt_np}
        for i in range(NCORES)
    ]
    r = run_bass_kernel_spmd(nc, in_maps, list(range(NCORES)),
                             trace=trace, trace_cores=trace_cores, **kw)
    out = np.concatenate([r.results[i]["out"] for i in range(NCORES)], axis=0)
    return out.reshape(N, C, H, W).astype(np.float32), r


def kernel(x: np.ndarray) -> np.ndarray:
    out, _ = run_sharded(np.asarray(x))
    return out


if __name__ == "__main__":
    # quick compile-only probe with a reduced plane count
    import sys
    import tempfile
    from concourse.bass_utils import compile_bir_kernel

    planes = int(sys.argv[1]) if len(sys.argv) > 1 else 2
    nc = _build_program(planes)
    d = tempfile.mkdtemp()
    print(f"compiling {planes}-plane program to {d} ...")
    neff = compile_bir_kernel(nc.to_json_bytes(), d, neff_name="probe.neff")
    print(f"COMPILE OK: {neff}")

